# revision 5
# baseline (speedup 1.0000x reference)
"""Causal self-attention on 8 TRN2 NeuronCores (Bass/Tile, SPMD).

Problem: y = CausalSelfAttention(x; Wqkv, bqkv, Wproj, bproj)
  x [B=4, T=2048, C=1024], H=16 heads, D=64.

Sharding: core c = (batch b = c//2, head-half hh = c%2). Each core computes
q/k/v for its 8 heads of its batch (Wqkv column-sharded), full causal
attention for those heads, and a partial output projection (Wproj
row-sharded). Host sums the two partials per batch and adds bproj.

Per-core kernel (all matmuls bf16 with fp32 PSUM accumulation):
  - q,k are produced d-major ([CL, T]) so QK^T needs no transposes;
    scores come out k-major [128 k, 512 q] per tile.
  - softmax skips the max-subtraction (scores are O(1) here; exp is safe)
    so it is a single fused exp on the Scalar engine; the causal mask is
    a bf16 multiply on the diagonal blocks only. Row sums come free from
    an extra ones-column appended to each per-head V tile (M=65 AV
    matmul), and 1/sum is broadcast across partitions via a tiny
    DRAM round-trip DMA.
  - Sub-diagonal k-tiles are skipped entirely (half the attention work).
"""

import math
from contextlib import ExitStack

import numpy as np
import ml_dtypes

import concourse.tile as tile
from concourse import bacc, mybir

BF16 = mybir.dt.bfloat16
F32 = mybir.dt.float32
NPBF16 = ml_dtypes.bfloat16

P = 128  # partitions / k-tile size
QB = 512  # q-block (matmul N; one fp32 PSUM bank)

B, T, C, H, D = 4, 2048, 1024, 16, 64
N_CORES = 8
HL = H // (N_CORES // B)  # heads per core (8)
CL = HL * D  # local head width (512)

# ---------------------------------------------------------------------------
# Per-core Bass program
# ---------------------------------------------------------------------------


def build_kernel(T=T, C=C, HL=HL, D=D, Cout=C):
    CL = HL * D
    n_ct = C // P
    n_mt = CL // P
    n_tt = T // P
    n_qb = T // QB
    n_hp = HL // 2
    dpb = QB // P
    n_cb = Cout // QB
    scale = 1.0 / math.sqrt(D)
    D1 = D + 1

    assert C % P == 0 and CL % P == 0 and T % QB == 0 and Cout % QB == 0
    assert HL % 2 == 0 and D == 64

    nc = bacc.Bacc("TRN2", target_bir_lowering=False, debug=False)
    xT = nc.dram_tensor("xT", [C, T], BF16, kind="ExternalInput")
    wq = nc.dram_tensor("wq", [C, CL], BF16, kind="ExternalInput")
    wk = nc.dram_tensor("wk", [C, CL], BF16, kind="ExternalInput")
    wv = nc.dram_tensor("wv", [C, CL], BF16, kind="ExternalInput")
    wp = nc.dram_tensor("wp", [CL, Cout], BF16, kind="ExternalInput")
    masks = nc.dram_tensor("masks", [QB, QB], BF16, kind="ExternalInput")
    out = nc.dram_tensor("out", [T, Cout], F32, kind="ExternalOutput")

    with tile.TileContext(nc) as tc, ExitStack() as ctx:
        persist = ctx.enter_context(tc.tile_pool(name="persist", bufs=1))
        psum_mm = ctx.enter_context(tc.tile_pool(name="psum_mm", bufs=4, space="PSUM"))
        psum_yt = ctx.enter_context(tc.tile_pool(name="psum_yt", bufs=2, space="PSUM"))
        ppool = ctx.enter_context(tc.tile_pool(name="ppool", bufs=4))
        npool = ctx.enter_context(tc.tile_pool(name="npool", bufs=4))
        dram = ctx.enter_context(tc.tile_pool(name="dram", bufs=4, space="DRAM"))

        # ---- persistent loads ----
        def load_tiles(src, n, rows, cols, tagp):
            ts = []
            for i in range(n):
                t = persist.tile([rows, cols], BF16, tag=f"{tagp}{i}", name=f"{tagp}{i}")
                nc.sync.dma_start(t[:], src[i * rows : (i + 1) * rows, :])
                ts.append(t)
            return ts

        xT_sb = load_tiles(xT, n_ct, P, T, "xT")
        wq_sb = load_tiles(wq, n_ct, P, CL, "wq")
        wk_sb = load_tiles(wk, n_ct, P, CL, "wk")
        wv_sb = load_tiles(wv, n_ct, P, CL, "wv")
        wp_sb = load_tiles(wp, n_mt, P, Cout, "wp")
        mask_sb = load_tiles(masks, dpb, P, QB, "mask")

        # ---- qkv projections ----
        q_d, k_d = [], []
        for name, w_sb, dst in (("q", wq_sb, q_d), ("k", wk_sb, k_d)):
            for m in range(n_mt):
                t = persist.tile([P, T], BF16, tag=f"{name}d{m}", name=f"{name}d{m}")
                dst.append(t)
                for b in range(n_qb):
                    ps = psum_mm.tile([P, QB], F32, tag="mm", name="mm")
                    for c in range(n_ct):
                        nc.tensor.matmul(
                            ps[:],
                            w_sb[c][:, m * P : (m + 1) * P],
                            xT_sb[c][:, b * QB : (b + 1) * QB],
                            start=(c == 0),
                            stop=(c == n_ct - 1),
                        )
                    nc.scalar.copy(t[:, b * QB : (b + 1) * QB], ps[:])

        # v with an interleaved ones column per head: v1 [T, HL*(D+1)]
        v1_sb = []
        for tt in range(n_tt):
            t = persist.tile([P, HL * D1], BF16, tag=f"v1_{tt}", name=f"v1_{tt}")
            v1_sb.append(t)
            ones_view = t[:].rearrange("p (h e) -> p h e", h=HL)[:, :, D : D + 1]
            nc.vector.memset(ones_view, 1.0)
            ps = psum_mm.tile([P, CL], F32, tag="mm", name="mm")
            for c in range(n_ct):
                nc.tensor.matmul(
                    ps[:],
                    xT_sb[c][:, tt * P : (tt + 1) * P],
                    wv_sb[c][:],
                    start=(c == 0),
                    stop=(c == n_ct - 1),
                )
            dst_view = t[:].rearrange("p (h e) -> p h e", h=HL)[:, :, 0:D]
            src_view = ps[:].rearrange("p (h e) -> p h e", h=HL)
            nc.vector.tensor_copy(dst_view, src_view)

        # ---- attention ----
        yT_sb = [
            persist.tile([P, T], BF16, tag=f"yT{m}", name=f"yT{m}")
            for m in range(n_mt)
        ]
        for hp in range(n_hp):
            for qb in range(n_qb):
                yts = [
                    psum_yt.tile([D1, QB], F32, tag="yt0", name="yt0"),
                    psum_yt.tile([D1, QB], F32, tag="yt1", name="yt1"),
                ]
                n_kt = dpb * qb + dpb
                for kt in range(n_kt):
                    for i in range(2):
                        h = 2 * hp + i
                        base = 64 * i
                        st = psum_mm.tile([P, QB], F32, tag="mm", name="mm")
                        nc.tensor.matmul(
                            st[:],
                            k_d[hp][base : base + 64, kt * P : (kt + 1) * P],
                            q_d[hp][base : base + 64, qb * QB : (qb + 1) * QB],
                            start=True,
                            stop=True,
                        )
                        pt = ppool.tile([P, QB], BF16, tag="pt", name="pt")
                        nc.scalar.activation(
                            pt[:], st[:], mybir.ActivationFunctionType.Exp, scale=scale
                        )
                        if kt >= dpb * qb:
                            nc.vector.tensor_mul(
                                pt[:], pt[:], mask_sb[kt - dpb * qb][:]
                            )
                        nc.tensor.matmul(
                            yts[i][:],
                            v1_sb[kt][:, h * D1 : (h + 1) * D1],
                            pt[:],
                            start=(kt == 0),
                            stop=(kt == n_kt - 1),
                        )
                for i in range(2):
                    yt = yts[i]
                    rec = npool.tile([1, QB], F32, tag="rec", name="rec")
                    nc.vector.reciprocal(rec[:], yt[D : D + 1, :])
                    ds = dram.tile([1, QB], F32, tag="ds", name="ds")
                    nc.sync.dma_start(ds[:], rec[:])
                    bc = npool.tile([64, QB], F32, tag="bc", name="bc")
                    nc.sync.dma_start(bc[:], ds[0:1, :].to_broadcast((64, QB)))
                    nc.vector.tensor_tensor(
                        yT_sb[hp][64 * i : 64 * i + 64, qb * QB : (qb + 1) * QB],
                        yt[0:D, :],
                        bc[:],
                        mybir.AluOpType.mult,
                    )

        # ---- output projection (partial over this core's heads) ----
        for tt in range(n_tt):
            for cb in range(n_cb):
                ps = psum_mm.tile([P, QB], F32, tag="mm", name="mm")
                for m in range(n_mt):
                    nc.tensor.matmul(
                        ps[:],
                        yT_sb[m][:, tt * P : (tt + 1) * P],
                        wp_sb[m][:, cb * QB : (cb + 1) * QB],
                        start=(m == 0),
                        stop=(m == n_mt - 1),
                    )
                ost = ppool.tile([P, QB], F32, tag="ostage", name="ostage")
                nc.scalar.copy(ost[:], ps[:])
                nc.sync.dma_start(
                    out[tt * P : (tt + 1) * P, cb * QB : (cb + 1) * QB], ost[:]
                )

    nc.compile()
    return nc


_PROGRAM_CACHE = {}


def _get_program(C_eff):
    key = C_eff
    if key not in _PROGRAM_CACHE:
        _PROGRAM_CACHE[key] = build_kernel(T=T, C=C_eff, HL=HL, D=D, Cout=C)
    return _PROGRAM_CACHE[key]


def _make_in_maps(x, Wqkv, bqkv):
    """Shard + cast inputs for the 8 cores. Returns (in_maps, C_eff)."""
    if np.any(bqkv):
        # Fold the qkv bias in as an extra contraction row (x gains a ones
        # column), zero-padded up to a multiple of 128.
        C_eff = ((C + 1 + P - 1) // P) * P
        Waug = np.zeros((C_eff, 3 * C), dtype=np.float32)
        Waug[:C] = Wqkv
        Waug[C] = bqkv
    else:
        C_eff = C
        Waug = Wqkv

    masks = (np.arange(QB)[:, None] <= np.arange(QB)[None, :]).astype(NPBF16)
    in_maps = []
    for core in range(N_CORES):
        b, hh = divmod(core, N_CORES // B)
        xT = np.zeros((C_eff, T), dtype=np.float32)
        xT[:C] = x[b].T
        if C_eff > C:
            xT[C] = 1.0
        c0 = hh * CL
        in_maps.append(
            {
                "xT": xT.astype(NPBF16),
                "wq": np.ascontiguousarray(Waug[:, 0 * C + c0 : 0 * C + c0 + CL]).astype(NPBF16),
                "wk": np.ascontiguousarray(Waug[:, 1 * C + c0 : 1 * C + c0 + CL]).astype(NPBF16),
                "wv": np.ascontiguousarray(Waug[:, 2 * C + c0 : 2 * C + c0 + CL]).astype(NPBF16),
                "wp": None,  # filled below (depends only on hh)
                "masks": masks,
            }
        )
    return in_maps, C_eff


def _run(x, Wqkv, bqkv, Wproj, bproj, trace=False):
    from concourse.bass_utils import run_bass_kernel_spmd

    in_maps, C_eff = _make_in_maps(x, Wqkv, bqkv)
    wp_by_hh = [
        np.ascontiguousarray(Wproj[hh * CL : (hh + 1) * CL, :]).astype(NPBF16)
        for hh in range(N_CORES // B)
    ]
    for core in range(N_CORES):
        in_maps[core]["wp"] = wp_by_hh[core % (N_CORES // B)]

    nc = _get_program(C_eff)
    res = run_bass_kernel_spmd(
        nc, in_maps, core_ids=list(range(N_CORES)), trace=trace
    )

    halves = N_CORES // B
    y = np.empty((B, T, C), dtype=np.float32)
    for b in range(B):
        acc = res.results[b * halves]["out"].astype(np.float32)
        for hh in range(1, halves):
            acc = acc + res.results[b * halves + hh]["out"]
        y[b] = acc + bproj.astype(np.float32)
    return y, res


def kernel(x, Wqkv, bqkv, Wproj, bproj):
    y, _ = _run(
        np.asarray(x, dtype=np.float32),
        np.asarray(Wqkv, dtype=np.float32),
        np.asarray(bqkv, dtype=np.float32),
        np.asarray(Wproj, dtype=np.float32),
        np.asarray(bproj, dtype=np.float32),
        trace=False,
    )
    return y


# revision 7
# speedup vs baseline: 1.4592x; 1.4592x over previous
"""Causal self-attention on 8 TRN2 NeuronCores (Bass/Tile, SPMD).

Problem: y = CausalSelfAttention(x; Wqkv, bqkv, Wproj, bproj)
  x [B=4, T=2048, C=1024], H=16 heads, D=64.

Sharding: core c = (batch b = c//2, head-half hh = c%2). Each core computes
q/k/v for its 8 heads of its batch (Wqkv column-sharded), full causal
attention for those heads, and a partial output projection (Wproj
row-sharded). Host sums the two partials per batch and adds bproj.

Per-core kernel (all matmuls bf16 with fp32 PSUM accumulation):
  - q,k are produced d-major ([CL, T]) so QK^T needs no transposes;
    scores come out k-major [128 k, 512 q] per tile.
  - softmax skips the max-subtraction (scores are O(1) here; exp is safe)
    so it is a single fused exp on the Scalar engine; the causal mask is
    a bf16 multiply on the diagonal blocks only. Row sums come free from
    an extra ones-column appended to each per-head V tile (M=65 AV
    matmul), and 1/sum is broadcast across partitions via a tiny
    DRAM round-trip DMA.
  - Sub-diagonal k-tiles are skipped entirely (half the attention work).
"""

import math
from contextlib import ExitStack

import numpy as np
import ml_dtypes

import concourse.tile as tile
from concourse import bacc, mybir

BF16 = mybir.dt.bfloat16
F32 = mybir.dt.float32
NPBF16 = ml_dtypes.bfloat16

P = 128  # partitions / k-tile size
QB = 512  # q-block (matmul N; one fp32 PSUM bank)

B, T, C, H, D = 4, 2048, 1024, 16, 64
N_CORES = 8
HL = H // (N_CORES // B)  # heads per core (8)
CL = HL * D  # local head width (512)

# ---------------------------------------------------------------------------
# Per-core Bass program
# ---------------------------------------------------------------------------


def build_kernel(T=T, C=C, HL=HL, D=D, Cout=C):
    CL = HL * D
    n_ct = C // P
    n_mt = CL // P
    n_tt = T // P
    n_qb = T // QB
    n_hp = HL // 2
    dpb = QB // P
    n_cb = Cout // QB
    scale = 1.0 / math.sqrt(D)
    D1 = D + 1
    n_sums = n_hp * n_qb * 2  # one softmax-denominator row per (head, q-block)

    assert C % P == 0 and CL % P == 0 and T % QB == 0 and Cout % QB == 0
    assert HL % 2 == 0 and D == 64 and n_mt == n_hp and n_sums <= P

    nc = bacc.Bacc("TRN2", target_bir_lowering=False, debug=False)
    xT = nc.dram_tensor("xT", [C, T], BF16, kind="ExternalInput")
    wq = nc.dram_tensor("wq", [C, CL], BF16, kind="ExternalInput")
    wk = nc.dram_tensor("wk", [C, CL], BF16, kind="ExternalInput")
    wv = nc.dram_tensor("wv", [C, CL], BF16, kind="ExternalInput")
    wp = nc.dram_tensor("wp", [CL, Cout], BF16, kind="ExternalInput")
    masks = nc.dram_tensor("masks", [QB, QB], BF16, kind="ExternalInput")
    out = nc.dram_tensor("out", [T, Cout], F32, kind="ExternalOutput")

    with tile.TileContext(nc) as tc, ExitStack() as ctx:
        persist = ctx.enter_context(tc.tile_pool(name="persist", bufs=1))
        # PSUM budget (8 banks): u512 4 x [128,512] + st2 2 x [128,1024]
        ps_u512 = ctx.enter_context(tc.tile_pool(name="ps_u512", bufs=4, space="PSUM"))
        ps_st2 = ctx.enter_context(tc.tile_pool(name="ps_st2", bufs=2, space="PSUM"))
        ppool = ctx.enter_context(tc.tile_pool(name="ppool", bufs=3))
        spool = ctx.enter_context(tc.tile_pool(name="spool", bufs=4))
        bcpool = ctx.enter_context(tc.tile_pool(name="bcpool", bufs=4))
        stage = ctx.enter_context(tc.tile_pool(name="stage", bufs=4))
        dram = ctx.enter_context(tc.tile_pool(name="dram", bufs=1, space="DRAM"))

        # ---- persistent loads ----
        def load_tiles(src, n, rows, cols, tagp):
            ts = []
            for i in range(n):
                t = persist.tile([rows, cols], BF16, tag=f"{tagp}{i}", name=f"{tagp}{i}")
                nc.sync.dma_start(t[:], src[i * rows : (i + 1) * rows, :])
                ts.append(t)
            return ts

        xT_sb = load_tiles(xT, n_ct, P, T, "xT")
        wq_sb = load_tiles(wq, n_ct, P, CL, "wq")
        wk_sb = load_tiles(wk, n_ct, P, CL, "wk")
        wv_sb = load_tiles(wv, n_ct, P, CL, "wv")
        wp_sb = load_tiles(wp, n_mt, P, Cout, "wp")
        mask_sb = load_tiles(masks, dpb, P, QB, "mask")

        sums_d = dram.tile([n_sums, QB], F32, tag="sums_d", name="sums_d")
        recips_d = dram.tile([n_sums, QB], F32, tag="recips_d", name="recips_d")

        # ---- v first (attention for every head pair needs all of v) ----
        # v with an interleaved ones column per head: v1 [T, HL*(D+1)]
        v1_sb = []
        for tt in range(n_tt):
            t = persist.tile([P, HL * D1], BF16, tag=f"v1_{tt}", name=f"v1_{tt}")
            v1_sb.append(t)
            ones_view = t[:].rearrange("p (h e) -> p h e", h=HL)[:, :, D : D + 1]
            nc.vector.memset(ones_view, 1.0)
            ps = ps_u512.tile([P, CL], F32, tag="u512", name="u512")
            for c in range(n_ct):
                nc.tensor.matmul(
                    ps[:],
                    xT_sb[c][:, tt * P : (tt + 1) * P],
                    wv_sb[c][:],
                    start=(c == 0),
                    stop=(c == n_ct - 1),
                )
            dst_view = t[:].rearrange("p (h e) -> p h e", h=HL)[:, :, 0:D]
            src_view = ps[:].rearrange("p (h e) -> p h e", h=HL)
            nc.vector.tensor_copy(dst_view, src_view)

        # ---- per head-pair: q/k projections then attention ----
        yT_sb = [
            persist.tile([P, T], BF16, tag=f"yT{m}", name=f"yT{m}")
            for m in range(n_mt)
        ]
        q_d = [None] * n_mt
        k_d = [None] * n_mt

        def sum_row(hp, qb, i):
            return (hp * n_qb + qb) * 2 + i

        for hp in range(n_hp):
            # q_d[hp], k_d[hp]: d-major [128, T] (head 2*hp rows 0:64,
            # head 2*hp+1 rows 64:128)
            for name, w_sb, dst in (("q", wq_sb, q_d), ("k", wk_sb, k_d)):
                t = persist.tile([P, T], BF16, tag=f"{name}d{hp}", name=f"{name}d{hp}")
                dst[hp] = t
                for b in range(n_qb):
                    ps = ps_u512.tile([P, QB], F32, tag="u512", name="u512")
                    for c in range(n_ct):
                        nc.tensor.matmul(
                            ps[:],
                            w_sb[c][:, hp * P : (hp + 1) * P],
                            xT_sb[c][:, b * QB : (b + 1) * QB],
                            start=(c == 0),
                            stop=(c == n_ct - 1),
                        )
                    nc.vector.tensor_copy(t[:, b * QB : (b + 1) * QB], ps[:])

            for qb in range(n_qb):
                yts = [
                    ps_u512.tile([D1, QB], F32, tag="u512", name="yt0"),
                    ps_u512.tile([D1, QB], F32, tag="u512", name="yt1"),
                ]
                n_kt = dpb * qb + dpb
                prev_pt = None

                def emit_av(kt, pt):
                    for i in range(2):
                        h = 2 * hp + i
                        nc.tensor.matmul(
                            yts[i][:],
                            v1_sb[kt][:, h * D1 : (h + 1) * D1],
                            pt[:, i * QB : (i + 1) * QB],
                            start=(kt == 0),
                            stop=(kt == n_kt - 1),
                        )

                for kt in range(n_kt):
                    # combined scores for both heads: [128 k, 1024]
                    st = ps_st2.tile([P, 2 * QB], F32, tag="st2", name="st2")
                    for i in range(2):
                        base = 64 * i
                        nc.tensor.matmul(
                            st[:, i * QB : (i + 1) * QB],
                            k_d[hp][base : base + 64, kt * P : (kt + 1) * P],
                            q_d[hp][base : base + 64, qb * QB : (qb + 1) * QB],
                            start=True,
                            stop=True,
                        )
                    pt = ppool.tile([P, 2 * QB], BF16, tag="pt", name="pt")
                    nc.scalar.activation(
                        pt[:], st[:], mybir.ActivationFunctionType.Exp, scale=scale
                    )
                    if kt >= dpb * qb:
                        m = mask_sb[kt - dpb * qb]
                        for i in range(2):
                            nc.vector.tensor_mul(
                                pt[:, i * QB : (i + 1) * QB],
                                pt[:, i * QB : (i + 1) * QB],
                                m[:],
                            )
                    # stagger: AV for the previous k-tile issues after this
                    # k-tile's scores, so the PE never queue-blocks on exp
                    if prev_pt is not None:
                        emit_av(*prev_pt)
                    prev_pt = (kt, pt)
                emit_av(*prev_pt)

                # epilogue: stash unnormalized y and the denominator row
                for i in range(2):
                    yt = yts[i]
                    nc.vector.tensor_copy(
                        yT_sb[hp][64 * i : 64 * i + 64, qb * QB : (qb + 1) * QB],
                        yt[0:D, :],
                    )
                    srow = spool.tile([1, QB], F32, tag="srow", name="srow")
                    nc.vector.tensor_copy(srow[:], yt[D : D + 1, :])
                    s = sum_row(hp, qb, i)
                    nc.sync.dma_start(sums_d[s : s + 1, :], srow[:])

        # ---- batched reciprocal of all softmax denominators ----
        allsums = stage.tile([n_sums, QB], F32, tag="allsums", name="allsums")
        nc.sync.dma_start(allsums[:], sums_d[:])
        allrec = stage.tile([n_sums, QB], F32, tag="allrec", name="allrec")
        nc.vector.reciprocal(allrec[:], allsums[:])
        nc.sync.dma_start(recips_d[:], allrec[:])

        # ---- normalize yT in place (partition-broadcast via DRAM DMA) ----
        for hp in range(n_hp):
            for qb in range(n_qb):
                bc = bcpool.tile([P, QB], F32, tag="bc", name="bc")
                for i in range(2):
                    s = sum_row(hp, qb, i)
                    nc.sync.dma_start(
                        bc[64 * i : 64 * i + 64, :],
                        recips_d[s : s + 1, :].to_broadcast((64, QB)),
                    )
                sl = yT_sb[hp][:, qb * QB : (qb + 1) * QB]
                nc.vector.tensor_mul(sl, sl, bc[:])

        # ---- output projection (partial over this core's heads) ----
        for tt in range(n_tt):
            for cb in range(n_cb):
                ps = ps_u512.tile([P, QB], F32, tag="u512", name="u512")
                for m in range(n_mt):
                    nc.tensor.matmul(
                        ps[:],
                        yT_sb[m][:, tt * P : (tt + 1) * P],
                        wp_sb[m][:, cb * QB : (cb + 1) * QB],
                        start=(m == 0),
                        stop=(m == n_mt - 1),
                    )
                ost = stage.tile([P, QB], F32, tag="ostage", name="ostage")
                nc.scalar.copy(ost[:], ps[:])
                nc.sync.dma_start(
                    out[tt * P : (tt + 1) * P, cb * QB : (cb + 1) * QB], ost[:]
                )

    nc.compile()
    return nc


_PROGRAM_CACHE = {}


def _get_program(C_eff):
    key = C_eff
    if key not in _PROGRAM_CACHE:
        _PROGRAM_CACHE[key] = build_kernel(T=T, C=C_eff, HL=HL, D=D, Cout=C)
    return _PROGRAM_CACHE[key]


def _make_in_maps(x, Wqkv, bqkv):
    """Shard + cast inputs for the 8 cores. Returns (in_maps, C_eff)."""
    if np.any(bqkv):
        # Fold the qkv bias in as an extra contraction row (x gains a ones
        # column), zero-padded up to a multiple of 128.
        C_eff = ((C + 1 + P - 1) // P) * P
        Waug = np.zeros((C_eff, 3 * C), dtype=np.float32)
        Waug[:C] = Wqkv
        Waug[C] = bqkv
    else:
        C_eff = C
        Waug = Wqkv

    masks = (np.arange(QB)[:, None] <= np.arange(QB)[None, :]).astype(NPBF16)
    in_maps = []
    for core in range(N_CORES):
        b, hh = divmod(core, N_CORES // B)
        xT = np.zeros((C_eff, T), dtype=np.float32)
        xT[:C] = x[b].T
        if C_eff > C:
            xT[C] = 1.0
        c0 = hh * CL
        in_maps.append(
            {
                "xT": xT.astype(NPBF16),
                "wq": np.ascontiguousarray(Waug[:, 0 * C + c0 : 0 * C + c0 + CL]).astype(NPBF16),
                "wk": np.ascontiguousarray(Waug[:, 1 * C + c0 : 1 * C + c0 + CL]).astype(NPBF16),
                "wv": np.ascontiguousarray(Waug[:, 2 * C + c0 : 2 * C + c0 + CL]).astype(NPBF16),
                "wp": None,  # filled below (depends only on hh)
                "masks": masks,
            }
        )
    return in_maps, C_eff


def _run(x, Wqkv, bqkv, Wproj, bproj, trace=False):
    from concourse.bass_utils import run_bass_kernel_spmd

    in_maps, C_eff = _make_in_maps(x, Wqkv, bqkv)
    wp_by_hh = [
        np.ascontiguousarray(Wproj[hh * CL : (hh + 1) * CL, :]).astype(NPBF16)
        for hh in range(N_CORES // B)
    ]
    for core in range(N_CORES):
        in_maps[core]["wp"] = wp_by_hh[core % (N_CORES // B)]

    nc = _get_program(C_eff)
    res = run_bass_kernel_spmd(
        nc, in_maps, core_ids=list(range(N_CORES)), trace=trace
    )

    halves = N_CORES // B
    y = np.empty((B, T, C), dtype=np.float32)
    for b in range(B):
        acc = res.results[b * halves]["out"].astype(np.float32)
        for hh in range(1, halves):
            acc = acc + res.results[b * halves + hh]["out"]
        y[b] = acc + bproj.astype(np.float32)
    return y, res


def kernel(x, Wqkv, bqkv, Wproj, bproj):
    y, _ = _run(
        np.asarray(x, dtype=np.float32),
        np.asarray(Wqkv, dtype=np.float32),
        np.asarray(bqkv, dtype=np.float32),
        np.asarray(Wproj, dtype=np.float32),
        np.asarray(bproj, dtype=np.float32),
        trace=False,
    )
    return y


# revision 11
# speedup vs baseline: 1.5931x; 1.0918x over previous
"""Causal self-attention on 8 TRN2 NeuronCores (Bass/Tile, SPMD).

Problem: y = CausalSelfAttention(x; Wqkv, bqkv, Wproj, bproj)
  x [B=4, T=2048, C=1024], H=16 heads, D=64.

Sharding: core c = (batch b = c//2, head-half hh = c%2). Each core computes
q/k/v for its 8 heads of its batch (Wqkv column-sharded), full causal
attention for those heads, and a partial output projection (Wproj
row-sharded). Host sums the two partials per batch and adds bproj.

Per-core kernel (all matmuls bf16 with fp32 PSUM accumulation):
  - q,k are produced d-major ([CL, T]) so QK^T needs no transposes;
    scores come out k-major [128 k, 512 q] per tile.
  - softmax skips the max-subtraction (scores are O(1) here; exp is safe)
    so it is a single fused exp on the Scalar engine; the causal mask is
    a bf16 multiply on the diagonal blocks only. Row sums come free from
    an extra ones-column appended to each per-head V tile (M=65 AV
    matmul), and 1/sum is broadcast across partitions via a tiny
    DRAM round-trip DMA.
  - Sub-diagonal k-tiles are skipped entirely (half the attention work).
"""

import math
from contextlib import ExitStack

import numpy as np
import ml_dtypes

import concourse.tile as tile
from concourse import bacc, mybir

BF16 = mybir.dt.bfloat16
F32 = mybir.dt.float32
NPBF16 = ml_dtypes.bfloat16

P = 128  # partitions / k-tile size
QB = 512  # q-block (matmul N; one fp32 PSUM bank)

B, T, C, H, D = 4, 2048, 1024, 16, 64
N_CORES = 8
HL = H // (N_CORES // B)  # heads per core (8)
CL = HL * D  # local head width (512)

# ---------------------------------------------------------------------------
# Per-core Bass program
# ---------------------------------------------------------------------------


def build_kernel(T=T, C=C, HL=HL, D=D, Cout=C):
    CL = HL * D
    n_ct = C // P
    n_mt = CL // P
    n_tt = T // P
    n_qb = T // QB
    n_hp = HL // 2
    dpb = QB // P
    n_cb = Cout // QB
    scale = 1.0 / math.sqrt(D)
    D1 = D + 1
    n_sums = n_hp * n_qb * 2  # one softmax-denominator row per (head, q-block)

    assert C % P == 0 and CL % P == 0 and T % QB == 0 and Cout % QB == 0
    assert HL % 2 == 0 and D == 64 and n_mt == n_hp and n_sums <= P

    nc = bacc.Bacc("TRN2", target_bir_lowering=False, debug=False)
    xT = nc.dram_tensor("xT", [C, T], BF16, kind="ExternalInput")
    wq = nc.dram_tensor("wq", [C, CL], BF16, kind="ExternalInput")
    wk = nc.dram_tensor("wk", [C, CL], BF16, kind="ExternalInput")
    wv = nc.dram_tensor("wv", [C, CL], BF16, kind="ExternalInput")
    wp = nc.dram_tensor("wp", [CL, Cout], BF16, kind="ExternalInput")
    masks = nc.dram_tensor("masks", [P, P], BF16, kind="ExternalInput")
    out = nc.dram_tensor("out", [T, Cout], F32, kind="ExternalOutput")

    with tile.TileContext(nc) as tc, ExitStack() as ctx:
        persist = ctx.enter_context(tc.tile_pool(name="persist", bufs=1))
        # PSUM budget (8 banks): u512 4 x [128,512] + st2 2 x [128,1024]
        ps_u512 = ctx.enter_context(tc.tile_pool(name="ps_u512", bufs=4, space="PSUM"))
        ps_st2 = ctx.enter_context(tc.tile_pool(name="ps_st2", bufs=2, space="PSUM"))
        ppool = ctx.enter_context(tc.tile_pool(name="ppool", bufs=3))
        spool = ctx.enter_context(tc.tile_pool(name="spool", bufs=4))
        bcpool = ctx.enter_context(tc.tile_pool(name="bcpool", bufs=4))
        stage = ctx.enter_context(tc.tile_pool(name="stage", bufs=4))
        dram = ctx.enter_context(tc.tile_pool(name="dram", bufs=1, space="DRAM"))

        # ---- persistent loads (v needs xT+wv first; wq/wk next; wp last) ----
        def load_tiles(src, n, rows, cols, tagp):
            ts = []
            for i in range(n):
                t = persist.tile([rows, cols], BF16, tag=f"{tagp}{i}", name=f"{tagp}{i}")
                nc.sync.dma_start(t[:], src[i * rows : (i + 1) * rows, :])
                ts.append(t)
            return ts

        xT_sb, wv_sb = [], []
        for i in range(n_ct):
            t = persist.tile([P, T], BF16, tag=f"xT{i}", name=f"xT{i}")
            nc.sync.dma_start(t[:], xT[i * P : (i + 1) * P, :])
            xT_sb.append(t)
            t = persist.tile([P, CL], BF16, tag=f"wv{i}", name=f"wv{i}")
            nc.sync.dma_start(t[:], wv[i * P : (i + 1) * P, :])
            wv_sb.append(t)
        trimask = persist.tile([P, P], BF16, tag="trimask", name="trimask")
        nc.sync.dma_start(trimask[:], masks[:])
        wq_sb = load_tiles(wq, n_ct, P, CL, "wq")
        wk_sb = load_tiles(wk, n_ct, P, CL, "wk")
        wp_sb = load_tiles(wp, n_mt, P, Cout, "wp")

        sums_d = dram.tile([n_sums, QB], F32, tag="sums_d", name="sums_d")
        recips_d = dram.tile([n_sums, QB], F32, tag="recips_d", name="recips_d")

        # ---- v first (attention for every head pair needs all of v) ----
        # v with an interleaved ones column per head: v1 [T, HL*(D+1)]
        v1_sb = []
        for tt in range(n_tt):
            t = persist.tile([P, HL * D1], BF16, tag=f"v1_{tt}", name=f"v1_{tt}")
            v1_sb.append(t)
            ones_view = t[:].rearrange("p (h e) -> p h e", h=HL)[:, :, D : D + 1]
            nc.vector.memset(ones_view, 1.0)
            ps = ps_u512.tile([P, CL], F32, tag="u512", name="u512")
            for c in range(n_ct):
                nc.tensor.matmul(
                    ps[:],
                    xT_sb[c][:, tt * P : (tt + 1) * P],
                    wv_sb[c][:],
                    start=(c == 0),
                    stop=(c == n_ct - 1),
                )
            dst_view = t[:].rearrange("p (h e) -> p h e", h=HL)[:, :, 0:D]
            src_view = ps[:].rearrange("p (h e) -> p h e", h=HL)
            nc.vector.tensor_copy(dst_view, src_view)

        # ---- per head-pair: q/k projections then attention ----
        yT_sb = [
            persist.tile([P, T], BF16, tag=f"yT{m}", name=f"yT{m}")
            for m in range(n_mt)
        ]
        q_d = [None] * n_mt
        k_d = [None] * n_mt

        def sum_row(hp, qb, i):
            return (hp * n_qb + qb) * 2 + i

        for hp in range(n_hp):
            # q_d[hp], k_d[hp]: d-major [128, T] (head 2*hp rows 0:64,
            # head 2*hp+1 rows 64:128)
            for name, w_sb, dst in (("q", wq_sb, q_d), ("k", wk_sb, k_d)):
                t = persist.tile([P, T], BF16, tag=f"{name}d{hp}", name=f"{name}d{hp}")
                dst[hp] = t
                for b in range(n_qb):
                    ps = ps_u512.tile([P, QB], F32, tag="u512", name="u512")
                    for c in range(n_ct):
                        nc.tensor.matmul(
                            ps[:],
                            w_sb[c][:, hp * P : (hp + 1) * P],
                            xT_sb[c][:, b * QB : (b + 1) * QB],
                            start=(c == 0),
                            stop=(c == n_ct - 1),
                        )
                    nc.vector.tensor_copy(t[:, b * QB : (b + 1) * QB], ps[:])

            for qb in range(n_qb):
                yts = [
                    ps_u512.tile([D1, QB], F32, tag="u512", name="yt0"),
                    ps_u512.tile([D1, QB], F32, tag="u512", name="yt1"),
                ]
                n_kt = dpb * qb + dpb
                prev_pt = None

                def emit_av(kt, pt):
                    # diagonal k-tiles only touch q-columns >= P*m
                    q0 = P * max(kt - dpb * qb, 0)
                    for i in range(2):
                        h = 2 * hp + i
                        nc.tensor.matmul(
                            yts[i][:, q0:QB],
                            v1_sb[kt][:, h * D1 : (h + 1) * D1],
                            pt[:, i * QB + q0 : (i + 1) * QB],
                            start=(kt == 0),
                            stop=(kt == n_kt - 1),
                            skip_group_check=True,
                        )

                for kt in range(n_kt):
                    m = kt - dpb * qb  # >=0: diagonal tile index
                    # combined scores for both heads: [128 k, 1024]
                    st = ps_st2.tile([P, 2 * QB], F32, tag="st2", name="st2")
                    for i in range(2):
                        base = 64 * i
                        nc.tensor.matmul(
                            st[:, i * QB : (i + 1) * QB],
                            k_d[hp][base : base + 64, kt * P : (kt + 1) * P],
                            q_d[hp][base : base + 64, qb * QB : (qb + 1) * QB],
                            start=True,
                            stop=True,
                        )
                    pt = ppool.tile([P, 2 * QB], BF16, tag="pt", name="pt")
                    if m <= 0:
                        nc.scalar.activation(
                            pt[:], st[:], mybir.ActivationFunctionType.Exp, scale=scale
                        )
                    else:
                        q0 = P * m
                        for i in range(2):
                            nc.scalar.activation(
                                pt[:, i * QB + q0 : (i + 1) * QB],
                                st[:, i * QB + q0 : (i + 1) * QB],
                                mybir.ActivationFunctionType.Exp,
                                scale=scale,
                            )
                    if m >= 0:
                        q0 = P * m
                        for i in range(2):
                            sl = pt[:, i * QB + q0 : i * QB + q0 + P]
                            nc.vector.tensor_mul(sl, sl, trimask[:])
                    # stagger: AV for the previous k-tile issues after this
                    # k-tile's scores, so the PE never queue-blocks on exp
                    if prev_pt is not None:
                        emit_av(*prev_pt)
                    prev_pt = (kt, pt)
                emit_av(*prev_pt)

                # epilogue: stash unnormalized y and the denominator row
                for i in range(2):
                    yt = yts[i]
                    nc.vector.tensor_copy(
                        yT_sb[hp][64 * i : 64 * i + 64, qb * QB : (qb + 1) * QB],
                        yt[0:D, :],
                    )
                    srow = spool.tile([1, QB], F32, tag="srow", name="srow")
                    nc.vector.tensor_copy(srow[:], yt[D : D + 1, :])
                    s = sum_row(hp, qb, i)
                    nc.sync.dma_start(sums_d[s : s + 1, :], srow[:])

            # ---- per-hp reciprocal + in-place normalize (overlaps next hp) ----
            ns_hp = 2 * n_qb
            s0 = sum_row(hp, 0, 0)
            allsums = stage.tile([ns_hp, QB], F32, tag="allsums", name="allsums")
            nc.sync.dma_start(allsums[:], sums_d[s0 : s0 + ns_hp, :])
            allrec = stage.tile([ns_hp, QB], F32, tag="allrec", name="allrec")
            nc.vector.reciprocal(allrec[:], allsums[:])
            nc.sync.dma_start(recips_d[s0 : s0 + ns_hp, :], allrec[:])
            for qb in range(n_qb):
                bc = bcpool.tile([P, QB], F32, tag="bc", name="bc")
                for i in range(2):
                    s = sum_row(hp, qb, i)
                    nc.sync.dma_start(
                        bc[64 * i : 64 * i + 64, :],
                        recips_d[s : s + 1, :].to_broadcast((64, QB)),
                    )
                sl = yT_sb[hp][:, qb * QB : (qb + 1) * QB]
                nc.vector.tensor_mul(sl, sl, bc[:])

        # ---- output projection (partial over this core's heads) ----
        for tt in range(n_tt):
            for cb in range(n_cb):
                ps = ps_u512.tile([P, QB], F32, tag="u512", name="u512")
                for m in range(n_mt):
                    nc.tensor.matmul(
                        ps[:],
                        yT_sb[m][:, tt * P : (tt + 1) * P],
                        wp_sb[m][:, cb * QB : (cb + 1) * QB],
                        start=(m == 0),
                        stop=(m == n_mt - 1),
                    )
                ost = stage.tile([P, QB], F32, tag="ostage", name="ostage")
                nc.scalar.copy(ost[:], ps[:])
                nc.sync.dma_start(
                    out[tt * P : (tt + 1) * P, cb * QB : (cb + 1) * QB], ost[:]
                )

    nc.compile()
    return nc


_PROGRAM_CACHE = {}


def _get_program(C_eff):
    key = C_eff
    if key not in _PROGRAM_CACHE:
        _PROGRAM_CACHE[key] = build_kernel(T=T, C=C_eff, HL=HL, D=D, Cout=C)
    return _PROGRAM_CACHE[key]


def _make_in_maps(x, Wqkv, bqkv):
    """Shard + cast inputs for the 8 cores. Returns (in_maps, C_eff)."""
    if np.any(bqkv):
        # Fold the qkv bias in as an extra contraction row (x gains a ones
        # column), zero-padded up to a multiple of 128.
        C_eff = ((C + 1 + P - 1) // P) * P
        Waug = np.zeros((C_eff, 3 * C), dtype=np.float32)
        Waug[:C] = Wqkv
        Waug[C] = bqkv
    else:
        C_eff = C
        Waug = Wqkv

    masks = (np.arange(P)[:, None] <= np.arange(P)[None, :]).astype(NPBF16)
    in_maps = []
    for core in range(N_CORES):
        b, hh = divmod(core, N_CORES // B)
        xT = np.zeros((C_eff, T), dtype=np.float32)
        xT[:C] = x[b].T
        if C_eff > C:
            xT[C] = 1.0
        c0 = hh * CL
        in_maps.append(
            {
                "xT": xT.astype(NPBF16),
                "wq": np.ascontiguousarray(Waug[:, 0 * C + c0 : 0 * C + c0 + CL]).astype(NPBF16),
                "wk": np.ascontiguousarray(Waug[:, 1 * C + c0 : 1 * C + c0 + CL]).astype(NPBF16),
                "wv": np.ascontiguousarray(Waug[:, 2 * C + c0 : 2 * C + c0 + CL]).astype(NPBF16),
                "wp": None,  # filled below (depends only on hh)
                "masks": masks,
            }
        )
    return in_maps, C_eff


def _run(x, Wqkv, bqkv, Wproj, bproj, trace=False):
    from concourse.bass_utils import run_bass_kernel_spmd

    in_maps, C_eff = _make_in_maps(x, Wqkv, bqkv)
    wp_by_hh = [
        np.ascontiguousarray(Wproj[hh * CL : (hh + 1) * CL, :]).astype(NPBF16)
        for hh in range(N_CORES // B)
    ]
    for core in range(N_CORES):
        in_maps[core]["wp"] = wp_by_hh[core % (N_CORES // B)]

    nc = _get_program(C_eff)
    res = run_bass_kernel_spmd(
        nc, in_maps, core_ids=list(range(N_CORES)), trace=trace
    )

    halves = N_CORES // B
    y = np.empty((B, T, C), dtype=np.float32)
    for b in range(B):
        acc = res.results[b * halves]["out"].astype(np.float32)
        for hh in range(1, halves):
            acc = acc + res.results[b * halves + hh]["out"]
        y[b] = acc + bproj.astype(np.float32)
    return y, res


def kernel(x, Wqkv, bqkv, Wproj, bproj):
    y, _ = _run(
        np.asarray(x, dtype=np.float32),
        np.asarray(Wqkv, dtype=np.float32),
        np.asarray(bqkv, dtype=np.float32),
        np.asarray(Wproj, dtype=np.float32),
        np.asarray(bproj, dtype=np.float32),
        trace=False,
    )
    return y
